# revision 9
# baseline (speedup 1.0000x reference)
"""Multi-head self-attention (B=2, S=2048, D=1024, H=16, causal) on 8 TRN2 cores.

Sharding: tensor-parallel over heads. Core c owns heads {2c, 2c+1}:
  - Wq/Wk/Wv column-sharded: core c gets columns [128c, 128c+128).
  - bf16 datapath end-to-end (PSUM stays f32): x^T, weights, Q^T/K^T/V,
    softmax weights, Z^T all bf16.  Matmuls run 1 cyc/row at any width.
  - Attention in transposed-scores layout: S^T[k, q] tiles; softmax
    denominators come free from a ones-column in V (row 64 of the AV psum).
    Causal masking folded into the scores matmul as a -BIG lower-triangular
    bias matmul on diagonal tiles; fully-masked blocks skipped.
  - Z^T normalized by per-q reciprocal (partition-broadcast on the idle Pool
    engine in phase A; DMA/PE broadcast in phase B to stay off the gpsimd
    queue once collectives are in flight).
  - Two AllToAlls, one per batch: batch-0's Z^T exchange + output-projection
    staging overlap batch-1's attention; only batch-1's exchange + projection
    remain exposed at the tail.  Core j receives all 1024 head-dims for its
    256 tokens of each batch; output projection needs full Wo rows and
    produces a disjoint [512, 1024] output slab per core (no all-reduce).
"""

import ml_dtypes
import numpy as np

import concourse.bass as bass
import concourse.mybir as mybir
import concourse.tile as tile
from concourse import bacc
from concourse.bass_utils import run_bass_kernel_spmd

N_CORES = 8
B, S, D = 2, 2048, 1024
H = 16
HD = D // H          # 64
BS = B * S           # 4096 flattened tokens
CD = 2 * HD          # 128 head-dims per core
BIG = 30000.0
SCALE = 1.0 / np.sqrt(HD)

F32 = mybir.dt.float32
BF16 = mybir.dt.bfloat16
EXP = mybir.ActivationFunctionType.Exp

_CACHE = {}


def build_nc(with_collective=True, reps=1):
    nc = bacc.Bacc("TRN2", target_bir_lowering=False, debug=False, num_devices=N_CORES)

    xT = nc.dram_tensor("xT", [D, BS], BF16, kind="ExternalInput").ap()
    wq = nc.dram_tensor("wq", [D, CD], BF16, kind="ExternalInput").ap()
    wk = nc.dram_tensor("wk", [D, CD], BF16, kind="ExternalInput").ap()
    wv = nc.dram_tensor("wv", [D, CD], BF16, kind="ExternalInput").ap()
    wo = nc.dram_tensor("wo", [D, D], BF16, kind="ExternalInput").ap()
    bo = nc.dram_tensor("bo", [1, D], F32, kind="ExternalInput").ap()
    tri01 = nc.dram_tensor("tri01", [128, 128], BF16, kind="ExternalInput").ap()
    ident = nc.dram_tensor("ident", [128, 128], BF16, kind="ExternalInput").ap()
    onesb = nc.dram_tensor("onesb", [128, 128], BF16, kind="ExternalInput").ap()
    out = nc.dram_tensor("out", [512, D], F32, kind="ExternalOutput").ap()

    with tile.TileContext(nc) as tc:
        with (
            tc.tile_pool(name="const", bufs=1) as constp,
            tc.tile_pool(name="persist", bufs=1) as persist,
            tc.tile_pool(name="xt", bufs=2) as xtp,
            tc.tile_pool(name="work", bufs=3) as work,
            tc.tile_pool(name="dram", bufs=1, space="DRAM") as dram,
        ):
            tri_sb = constp.tile([128, 128], BF16)
            ident_sb = constp.tile([128, 128], BF16)
            ones_sb = constp.tile([128, 128], BF16)
            cc_in = [dram.tile([8, 128, 256], BF16, name=f"ccin{b}") for b in (0, 1)]
            cc_out = [dram.tile([8, 128, 256], BF16, name=f"ccout{b}") for b in (0, 1)]
            xTr = xT.rearrange("(e p) s -> p e s", p=128)

            for _rep in range(reps):
                _body(nc, tc, constp, persist, xtp, work, dram,
                      xTr, wq, wk, wv, wo, bo, out,
                      tri_sb, ident_sb, ones_sb, cc_in, cc_out,
                      with_collective, (tri01, ident, onesb),
                      first=(_rep == 0))

    nc.compile()
    return nc


def _body(nc, tc, constp, persist, xtp, work, dram,
          xTr, wq, wk, wv, wo, bo, out,
          tri_sb, ident_sb, ones_sb, cc_in, cc_out,
          with_collective, const_srcs, first=True):
    # ---- projection weights ----
    wq_sb = constp.tile([128, 8, CD], BF16, tag="wq", name="wq_sb")
    wk_sb = constp.tile([128, 8, CD], BF16, tag="wk", name="wk_sb")
    wv_sb = constp.tile([128, 8, CD], BF16, tag="wv", name="wv_sb")
    wqr = wq.rearrange("(e p) c -> p e c", p=128)
    nc.sync.dma_start(wq_sb[:, 0:1, :], wqr[:, 0:1, :])
    nc.sync.dma_start(wq_sb[:, 1:4, :], wqr[:, 1:4, :])

    # ---- persistent activations ----
    qt_sb = persist.tile([128, BS], BF16, tag="qt", name="qt_sb", bufs=2)
    kt_sb = persist.tile([128, BS], BF16, tag="kt", name="kt_sb", bufs=2)
    v_sb = persist.tile([128, 32, 130], BF16, tag="v", name="v_sb")
    bo_bc = persist.tile([128, D], F32, tag="bobc", name="bo_bc")
    wo_sb = persist.tile([128, 8, D], BF16, tag="wo", name="wo_sb")

    P = {}  # current-phase psum pools

    def v_transposes(sc, vt_t):
        for st in range(4):
            tt = 4 * sc + st
            v_ps = P["pp"].tile([128, 128], BF16, tag="p", name=f"vtp{sc}{st}")
            nc.tensor.transpose(
                v_ps[:], vt_t[:, 128 * st:128 * st + 128], ident_sb[:],
            )
            nc.vector.tensor_copy(v_sb[:, tt, 0:64], v_ps[:, 0:64])
            nc.vector.tensor_copy(v_sb[:, tt, 65:129], v_ps[:, 64:128])

    def proj_parts(sc):
        """Yield fine-grained projection closures for one 512-token chunk."""
        sl = bass.ts(sc, 512)
        state = {}

        def load():
            xt_t = xtp.tile([128, 8, 512], BF16, tag="xt", name=f"xt{sc}")
            if sc == 0:
                nc.sync.dma_start(xt_t[:, 0:1, :], xTr[:, 0:1, sl])
                nc.sync.dma_start(xt_t[:, 1:4, :], xTr[:, 1:4, sl])
                nc.sync.dma_start(wq_sb[:, 4:8, :], wqr[:, 4:8, :])
            else:
                nc.sync.dma_start(xt_t[:, 0:4, :], xTr[:, 0:4, sl])
            nc.sync.dma_start(xt_t[:, 4:8, :], xTr[:, 4:8, sl])
            if sc == 0:
                # defer K/V weight loads so the first Q matmuls start sooner
                nc.sync.dma_start(
                    wk_sb[:], wk.rearrange("(e p) c -> p e c", p=128))
                nc.sync.dma_start(
                    wv_sb[:], wv.rearrange("(e p) c -> p e c", p=128))
                if first:
                    tri_d, ident_d, onesb_d = const_srcs
                    nc.sync.dma_start(tri_sb[:], tri_d)
                    nc.sync.dma_start(ident_sb[:], ident_d)
                    nc.sync.dma_start(ones_sb[:], onesb_d)
                nc.vector.tensor_copy(v_sb[:, :, 64], ones_sb[:, 0:32])
                nc.vector.tensor_copy(v_sb[:, :, 129], ones_sb[:, 0:32])
            state["xt"] = xt_t
            state["vt"] = xtp.tile([128, 512], BF16, tag="vtc", name=f"vtc{sc}")

        def group(w_sb, o_ap_fn, name):
            def run():
                p_ps = P["pp"].tile([128, 512], F32, tag="p", name=f"pp{sc}{name}")
                for e in range(8):
                    nc.tensor.matmul(
                        p_ps[:], w_sb[:, e, :], state["xt"][:, e, :],
                        start=(e == 0), stop=(e == 7),
                    )
                nc.vector.tensor_copy(o_ap_fn(), p_ps[:])
            return run

        yield load
        yield group(wq_sb, lambda: qt_sb[:, sl], "q")
        yield group(wk_sb, lambda: kt_sb[:, sl], "k")
        yield group(wv_sb, lambda: state["vt"][:], "v")
        yield lambda: v_transposes(sc, state["vt"][:])

    def proj_chunk(sc):
        for part in proj_parts(sc):
            part()

    def attn_chunk_beats(b, m, stream):
        """Yield one closure per beat; caller weaves streams together."""
        q0 = 2048 * b + 512 * m
        last_t = 4 * m + 3
        state = {}

        def beat(t):
            if t == 0:
                state["z"] = [
                    P["pz"].tile([65, 512], F32, tag=f"z{stream}{h}",
                                 name=f"z{b}{m}{h}", bufs=1)
                    for h in (0, 1)
                ]
            z_ps = state["z"]

            def av(ta, pt_sb):
                joa = max(0, 128 * (ta - 4 * m))
                for h in (0, 1):
                    nc.tensor.matmul(
                        z_ps[h][:, joa:512],
                        v_sb[:, 16 * b + ta, 65 * h:65 * h + 65],
                        pt_sb[:, 512 * h + joa:512 * h + 512],
                        start=(ta == 0), stop=(ta == last_t),
                    )

            k0 = 2048 * b + 128 * t
            jo = max(0, 128 * (t - 4 * m))
            pt_sb = work.tile([128, 1024], BF16, tag="pt", name=f"pt{b}{m}{t}", bufs=6)
            s_ps = P["ps"].tile([128, 1024], F32, tag="s", name=f"s{b}{m}{t}")
            for h in (0, 1):
                hsl = slice(64 * h, 64 * h + 64)
                nc.tensor.matmul(
                    s_ps[:, 512 * h + jo:512 * h + 512],
                    kt_sb[hsl, k0:k0 + 128],
                    qt_sb[hsl, q0 + jo:q0 + 512],
                    start=True, stop=True,
                )
            nc.scalar.activation(
                pt_sb[:].rearrange("p (h w) -> p h w", h=2)[:, :, jo:512],
                s_ps[:].rearrange("p (h w) -> p h w", h=2)[:, :, jo:512],
                EXP, scale=float(SCALE),
            )
            if t >= 4 * m:
                # zero the strictly-masked (k > q) triangle of the diagonal
                # 128-block post-exp; replaces a -BIG bias matmul on PE
                ptd = pt_sb[:].rearrange("p (h w) -> p h w", h=2)[:, :, jo:jo + 128]
                trib = tri_sb[:].rearrange("p (o w) -> p o w", o=1)
                nc.vector.tensor_mul(ptd, ptd, trib.broadcast_to([128, 2, 128]))
            pend = state.pop("pend", None)
            if pend is not None:
                av(*pend)
            state["pend"] = (t, pt_sb)
            if t == last_t:
                av(*state.pop("pend"))
                _norm(b, m, z_ps, bcast="pool")

        for t in range(last_t + 1):
            yield lambda t=t: beat(t)

    def _norm(b, m, z_ps, bcast="pool"):
        # normalize and stage for all-to-all; copy psum out (incl. denom row)
        # immediately to release the z banks, then finish from SBUF
        zcp = [work.tile([65, 512], F32, tag=f"zc{h}", name=f"zc{b}{m}{h}", bufs=2)
               for h in (0, 1)]
        for h in (0, 1):
            nc.vector.tensor_copy(zcp[h][:], z_ps[h][:])
        zt_sb = work.tile([128, 512], BF16, tag="zt", name=f"zt{b}{m}", bufs=2)
        for h in (0, 1):
            recip = work.tile([1, 512], F32, tag="rc", name=f"rc{b}{m}{h}", bufs=2)
            nc.vector.reciprocal(recip[0:1, :], zcp[h][64:65, :])
            bc_sb = work.tile([64, 512], F32, tag="bc", name=f"bcs{b}{m}{h}", bufs=2)
            if bcast == "pool":
                nc.gpsimd.partition_broadcast(bc_sb[:], recip[0:1, :])
            elif bcast == "pe":
                recb = work.tile([1, 512], BF16, tag="rcb", name=f"rb{b}{m}{h}", bufs=2)
                nc.vector.tensor_copy(recb[0:1, :], recip[0:1, :])
                bc_ps = P["ps"].tile([64, 512], F32, tag="s", name=f"bcp{b}{m}{h}")
                nc.tensor.matmul(
                    bc_ps[:], ones_sb[0:1, 0:64], recb[0:1, :],
                    start=True, stop=True,
                )
                nc.vector.tensor_copy(bc_sb[:], bc_ps[:])
            else:
                r_dram = dram.tile([1, 512], F32, tag="rd", name=f"rd{b}{m}{h}", bufs=2)
                nc.sync.dma_start(r_dram[:], recip[0:1, :])
                nc.sync.dma_start(bc_sb[:], r_dram.broadcast_to([64, 512]))
            nc.vector.tensor_mul(
                zt_sb[64 * h:64 * h + 64, :], zcp[h][0:64, :], bc_sb[:]
            )
        nc.sync.dma_start(cc_in[b][2 * m], zt_sb[:, 0:256])
        nc.sync.dma_start(cc_in[b][2 * m + 1], zt_sb[:, 256:512])

    def exchange(i):
        if with_collective:
            nc.gpsimd.collective_compute(
                "AllToAll",
                mybir.AluOpType.bypass,
                replica_groups=[list(range(N_CORES))],
                ins=[cc_in[i].opt()],
                outs=[cc_out[i].opt()],
            )
        else:
            nc.sync.dma_start(cc_out[i][:], cc_in[i][:])

    def load_zt2(i, zt2_sb):
        ccr = cc_out[i].rearrange("j p s -> p j s")
        nc.sync.dma_start(zt2_sb[:, 0:4, :], ccr[:, 0:4, :])
        nc.sync.dma_start(zt2_sb[:, 4:8, :], ccr[:, 4:8, :])

    def out_proj_parts(tag, zt2_sb, sts, rows, pool_fn, ptag=None):
        # O[tok, :] = Z^T.T @ Wo + bo, one closure per 128-token tile
        def part(st, r0):
            def run():
                o_sb = work.tile([128, 1024], F32, tag="o", name=f"os{tag}{st}",
                                 bufs=2)
                for e in (0, 1):
                    o_ps = pool_fn().tile([128, 512], F32,
                                          tag=(ptag or f"o{e}"),
                                          name=f"o{tag}{st}{e}")
                    for i in range(8):
                        nc.tensor.matmul(
                            o_ps[:],
                            zt2_sb[:, i, bass.ts(st, 128)],
                            wo_sb[:, i, bass.ts(e, 512)],
                            start=(i == 0), stop=(i == 7),
                        )
                    nc.vector.tensor_add(
                        o_sb[:, bass.ts(e, 512)], o_ps[:], bo_bc[:, bass.ts(e, 512)]
                    )
                nc.sync.dma_start(out[r0:r0 + 128, :], o_sb[:])
            return run
        return [part(st, r) for st, r in zip(sts, rows)]

    def weave(tasks_a, tasks_b, fillers, boost=0):
        """Round-robin beats from attention streams, sprinkling filler
        closures (projection work) between rounds; `boost` rounds take two
        fillers so early chunks stay ahead of the beats that need them."""
        ia = iter(tasks_a)
        ib = iter(tasks_b)
        fi = iter(fillers)
        done_a = done_b = False
        rnd = 0
        while not (done_a and done_b):
            try:
                next(ia)()
            except StopIteration:
                done_a = True
            try:
                next(ib)()
            except StopIteration:
                done_b = True
            for _ in range(2 if rnd < boost else 1):
                f = next(fi, None)
                if f is not None:
                    f()
            rnd += 1
        for f in fi:
            f()

    def proj_fillers_a():
        for sc in range(1, 8):
            yield from proj_parts(sc)

    # phase A: projections + batch-0 attention (single stream; z uses
    # alternating tag pairs so chunk boundaries overlap)
    with (
        tc.tile_pool(name="ppA", bufs=2, space="PSUM") as ppA,
        tc.tile_pool(name="psA", bufs=2, space="PSUM") as psA,
        tc.tile_pool(name="pzA", bufs=1, space="PSUM") as pzA,
    ):
        P["pp"] = ppA
        P["ps"] = psA
        P["pz"] = pzA
        proj_chunk(0)
        beats_b0 = (
            list(attn_chunk_beats(0, 0, "A")) + list(attn_chunk_beats(0, 1, "A"))
            + list(attn_chunk_beats(0, 2, "A")) + list(attn_chunk_beats(0, 3, "A"))
        )
        weave(beats_b0, [], list(proj_fillers_a()), boost=10)

    # batch-0 Z^T exchange flies while batch-1 attention computes
    exchange(0)

    zt2a = persist.tile([128, 8, 256], BF16, tag="zt2a", name="zt2a")
    zt2b = persist.tile([128, 8, 256], BF16, tag="zt2b", name="zt2b")
    wor = wo.rearrange("(i p) e -> p i e", p=128)

    # phase B: batch-1 attention, two balanced streams
    with (
        tc.tile_pool(name="psB", bufs=2, space="PSUM") as psB,
        tc.tile_pool(name="pzB", bufs=1, space="PSUM") as pzB,
    ):
        P["pp"] = psB
        P["ps"] = psB
        P["pz"] = pzB
        fillers_b = [
            lambda: nc.sync.dma_start(wo_sb[:, 0:4, :], wor[:, 0:4, :]),
            lambda: nc.sync.dma_start(wo_sb[:, 4:8, :], wor[:, 4:8, :]),
            lambda: nc.sync.dma_start(bo_bc[:], bo.broadcast_to([128, D])),
            None, None, None, None, None,
            lambda: load_zt2(0, zt2a),
        ]
        weave(
            list(attn_chunk_beats(1, 0, "A")) + list(attn_chunk_beats(1, 3, "A")),
            list(attn_chunk_beats(1, 1, "B")) + list(attn_chunk_beats(1, 2, "B")),
            fillers_b,
        )

    # batch-1 exchange overlaps batch-0's output projection
    exchange(1)
    with tc.tile_pool(name="poC", bufs=1, space="PSUM") as poC:
        for part in out_proj_parts("a", zt2a, (0, 1), (0, 128), lambda: poC):
            part()
        load_zt2(1, zt2b)
        for part in out_proj_parts("b", zt2b, (0, 1), (256, 384), lambda: poC):
            part()


def _prep_inputs(inputs, Wq, Wk, Wv, Wo, bo):
    bf = ml_dtypes.bfloat16
    x = np.asarray(inputs, dtype=np.float32).reshape(BS, D)
    xT = np.ascontiguousarray(x.T.astype(bf))
    Wq = np.asarray(Wq, dtype=np.float32)
    Wk = np.asarray(Wk, dtype=np.float32)
    Wv = np.asarray(Wv, dtype=np.float32)
    Wo = np.ascontiguousarray(np.asarray(Wo, dtype=np.float32).astype(bf))
    bo = np.asarray(bo, dtype=np.float32).reshape(1, D)
    tri01 = np.triu(np.ones((128, 128), dtype=np.float32), k=0).astype(bf)
    ident = np.eye(128, dtype=np.float32).astype(bf)
    onesb = np.ones((128, 128), dtype=np.float32).astype(bf)
    in_maps = []
    for c in range(N_CORES):
        csl = slice(CD * c, CD * (c + 1))
        in_maps.append({
            "xT": xT,
            "wq": np.ascontiguousarray(Wq[:, csl].astype(bf)),
            "wk": np.ascontiguousarray(Wk[:, csl].astype(bf)),
            "wv": np.ascontiguousarray(Wv[:, csl].astype(bf)),
            "wo": Wo,
            "bo": bo,
            "tri01": tri01,
            "ident": ident,
            "onesb": onesb,
        })
    return in_maps


def kernel(inputs, Wq, Wk, Wv, Wo, bo):
    if "nc" not in _CACHE:
        _CACHE["nc"] = build_nc()
    nc = _CACHE["nc"]
    in_maps = _prep_inputs(inputs, Wq, Wk, Wv, Wo, bo)
    res = None
    for attempt in range(3):
        try:
            res = run_bass_kernel_spmd(nc, in_maps, core_ids=list(range(N_CORES)))
            break
        except Exception:
            if attempt == 2:
                raise
            import time as _time

            _time.sleep(5.0)
    # core c owns tokens [256c, 256c+256) of each batch
    full = np.empty((B, S, D), dtype=np.float32)
    for c in range(N_CORES):
        slab = res.results[c]["out"]
        full[0, 256 * c:256 * c + 256] = slab[0:256]
        full[1, 256 * c:256 * c + 256] = slab[256:512]
    return full


# revision 10
# speedup vs baseline: 1.4333x; 1.4333x over previous
"""Multi-head self-attention (B=2, S=2048, D=1024, H=16, causal) on 8 TRN2 cores.

Sharding: tensor-parallel over heads. Core c owns heads {2c, 2c+1}:
  - Wq/Wk/Wv column-sharded: core c gets columns [128c, 128c+128).
  - bf16 datapath end-to-end (PSUM stays f32): x^T, weights, Q^T/K^T/V,
    softmax weights, Z^T all bf16.  Matmuls run 1 cyc/row at any width.
  - Attention in transposed-scores layout: S^T[k, q] tiles; softmax
    denominators come free from a ones-column in V (row 64 of the AV psum).
    Causal masking folded into the scores matmul as a -BIG lower-triangular
    bias matmul on diagonal tiles; fully-masked blocks skipped.
  - Z^T normalized by per-q reciprocal (partition-broadcast on the idle Pool
    engine in phase A; DMA/PE broadcast in phase B to stay off the gpsimd
    queue once collectives are in flight).
  - Two AllToAlls, one per batch: batch-0's Z^T exchange + output-projection
    staging overlap batch-1's attention; only batch-1's exchange + projection
    remain exposed at the tail.  Core j receives all 1024 head-dims for its
    256 tokens of each batch; output projection needs full Wo rows and
    produces a disjoint [512, 1024] output slab per core (no all-reduce).
"""

import ml_dtypes
import numpy as np

import concourse.bass as bass
import concourse.mybir as mybir
import concourse.tile as tile
from concourse import bacc
from concourse.bass_utils import run_bass_kernel_spmd

N_CORES = 8
B, S, D = 2, 2048, 1024
H = 16
HD = D // H          # 64
BS = B * S           # 4096 flattened tokens
CD = 2 * HD          # 128 head-dims per core
BIG = 30000.0
SCALE = 1.0 / np.sqrt(HD)

F32 = mybir.dt.float32
BF16 = mybir.dt.bfloat16
EXP = mybir.ActivationFunctionType.Exp

_CACHE = {}


def build_nc(with_collective=True, reps=1):
    nc = bacc.Bacc("TRN2", target_bir_lowering=False, debug=False, num_devices=N_CORES)

    xT = nc.dram_tensor("xT", [D, BS], BF16, kind="ExternalInput").ap()
    wq = nc.dram_tensor("wq", [128, 8 * CD], BF16, kind="ExternalInput").ap()
    wk = nc.dram_tensor("wk", [128, 8 * CD], BF16, kind="ExternalInput").ap()
    wv = nc.dram_tensor("wv", [128, 8 * CD], BF16, kind="ExternalInput").ap()
    wo = nc.dram_tensor("wo", [128, 8 * D], BF16, kind="ExternalInput").ap()
    bo = nc.dram_tensor("bo", [1, D], F32, kind="ExternalInput").ap()
    tri01 = nc.dram_tensor("tri01", [128, 128], BF16, kind="ExternalInput").ap()
    ident = nc.dram_tensor("ident", [128, 128], BF16, kind="ExternalInput").ap()
    onesb = nc.dram_tensor("onesb", [128, 128], BF16, kind="ExternalInput").ap()
    out = nc.dram_tensor("out", [512, D], F32, kind="ExternalOutput").ap()

    with tile.TileContext(nc) as tc:
        with (
            tc.tile_pool(name="const", bufs=1) as constp,
            tc.tile_pool(name="persist", bufs=1) as persist,
            tc.tile_pool(name="xt", bufs=2) as xtp,
            tc.tile_pool(name="work", bufs=3) as work,
            tc.tile_pool(name="dram", bufs=1, space="DRAM") as dram,
        ):
            tri_sb = constp.tile([128, 128], BF16)
            ident_sb = constp.tile([128, 128], BF16)
            ones_sb = constp.tile([128, 128], BF16)
            cc_in = [dram.tile([8, 128, 256], BF16, name=f"ccin{b}") for b in (0, 1)]
            cc_out = [dram.tile([8, 128, 256], BF16, name=f"ccout{b}") for b in (0, 1)]
            xTr = xT.rearrange("(e p) s -> p e s", p=128)

            for _rep in range(reps):
                _body(nc, tc, constp, persist, xtp, work, dram,
                      xTr, wq, wk, wv, wo, bo, out,
                      tri_sb, ident_sb, ones_sb, cc_in, cc_out,
                      with_collective, (tri01, ident, onesb),
                      first=(_rep == 0))

    nc.compile()
    return nc


def _body(nc, tc, constp, persist, xtp, work, dram,
          xTr, wq, wk, wv, wo, bo, out,
          tri_sb, ident_sb, ones_sb, cc_in, cc_out,
          with_collective, const_srcs, first=True):
    # ---- projection weights ----
    wq_sb = constp.tile([128, 8, CD], BF16, tag="wq", name="wq_sb")
    wk_sb = constp.tile([128, 8, CD], BF16, tag="wk", name="wk_sb")
    wv_sb = constp.tile([128, 8, CD], BF16, tag="wv", name="wv_sb")
    wqr = wq.rearrange("p (e c) -> p e c", e=8)
    nc.sync.dma_start(wq_sb[:, 0:1, :], wqr[:, 0:1, :])
    nc.sync.dma_start(wq_sb[:, 1:4, :], wqr[:, 1:4, :])

    # ---- persistent activations ----
    qt_sb = persist.tile([128, BS], BF16, tag="qt", name="qt_sb")
    kt_sb = persist.tile([128, BS], BF16, tag="kt", name="kt_sb")
    v_sb = persist.tile([128, 32, 130], BF16, tag="v", name="v_sb")
    bo_bc = persist.tile([128, D], F32, tag="bobc", name="bo_bc")
    wo_sb = persist.tile([128, 8, D], BF16, tag="wo", name="wo_sb")

    P = {}  # current-phase psum pools

    def v_transposes(sc, vt_t):
        for st in range(4):
            tt = 4 * sc + st
            v_ps = P["pp"].tile([128, 128], BF16, tag="p", name=f"vtp{sc}{st}")
            nc.tensor.transpose(
                v_ps[:], vt_t[:, 128 * st:128 * st + 128], ident_sb[:],
            )
            nc.vector.tensor_copy(v_sb[:, tt, 0:64], v_ps[:, 0:64])
            nc.vector.tensor_copy(v_sb[:, tt, 65:129], v_ps[:, 64:128])

    def proj_parts(sc):
        """Yield fine-grained projection closures for one 512-token chunk."""
        sl = bass.ts(sc, 512)
        state = {}

        def load():
            xt_t = xtp.tile([128, 8, 512], BF16, tag="xt", name=f"xt{sc}")
            if sc == 0:
                nc.sync.dma_start(xt_t[:, 0:1, :], xTr[:, 0:1, sl])
                nc.sync.dma_start(xt_t[:, 1:4, :], xTr[:, 1:4, sl])
                nc.sync.dma_start(wq_sb[:, 4:8, :], wqr[:, 4:8, :])
            else:
                nc.sync.dma_start(xt_t[:, 0:4, :], xTr[:, 0:4, sl])
            nc.sync.dma_start(xt_t[:, 4:8, :], xTr[:, 4:8, sl])
            if sc == 0:
                # defer K/V weight loads so the first Q matmuls start sooner
                nc.sync.dma_start(
                    wk_sb[:], wk.rearrange("p (e c) -> p e c", e=8))
                nc.sync.dma_start(
                    wv_sb[:], wv.rearrange("p (e c) -> p e c", e=8))
                if first:
                    tri_d, ident_d, onesb_d = const_srcs
                    nc.sync.dma_start(tri_sb[:], tri_d)
                    nc.sync.dma_start(ident_sb[:], ident_d)
                    nc.sync.dma_start(ones_sb[:], onesb_d)
                nc.vector.tensor_copy(v_sb[:, :, 64], ones_sb[:, 0:32])
                nc.vector.tensor_copy(v_sb[:, :, 129], ones_sb[:, 0:32])
            state["xt"] = xt_t
            state["vt"] = xtp.tile([128, 512], BF16, tag="vtc", name=f"vtc{sc}")

        def group(w_sb, o_ap_fn, name):
            def run():
                p_ps = P["pp"].tile([128, 512], F32, tag="p", name=f"pp{sc}{name}")
                for e in range(8):
                    nc.tensor.matmul(
                        p_ps[:], w_sb[:, e, :], state["xt"][:, e, :],
                        start=(e == 0), stop=(e == 7),
                    )
                nc.vector.tensor_copy(o_ap_fn(), p_ps[:])
            return run

        yield load
        yield group(wq_sb, lambda: qt_sb[:, sl], "q")
        yield group(wk_sb, lambda: kt_sb[:, sl], "k")
        yield group(wv_sb, lambda: state["vt"][:], "v")
        yield lambda: v_transposes(sc, state["vt"][:])

    def proj_chunk(sc):
        for part in proj_parts(sc):
            part()

    def attn_chunk_beats(b, m, stream):
        """Yield one closure per beat; caller weaves streams together."""
        q0 = 2048 * b + 512 * m
        last_t = 4 * m + 3
        state = {}

        def beat(t):
            if t == 0:
                state["z"] = [
                    P["pz"].tile([65, 512], F32, tag=f"z{stream}{h}",
                                 name=f"z{b}{m}{h}", bufs=1)
                    for h in (0, 1)
                ]
            z_ps = state["z"]

            def av(ta, pt_sb):
                joa = max(0, 128 * (ta - 4 * m))
                for h in (0, 1):
                    nc.tensor.matmul(
                        z_ps[h][:, joa:512],
                        v_sb[:, 16 * b + ta, 65 * h:65 * h + 65],
                        pt_sb[:, 512 * h + joa:512 * h + 512],
                        start=(ta == 0), stop=(ta == last_t),
                    )

            k0 = 2048 * b + 128 * t
            jo = max(0, 128 * (t - 4 * m))
            pt_sb = work.tile([128, 1024], BF16, tag="pt", name=f"pt{b}{m}{t}", bufs=6)
            s_ps = P["ps"].tile([128, 1024], F32, tag="s", name=f"s{b}{m}{t}")
            for h in (0, 1):
                hsl = slice(64 * h, 64 * h + 64)
                nc.tensor.matmul(
                    s_ps[:, 512 * h + jo:512 * h + 512],
                    kt_sb[hsl, k0:k0 + 128],
                    qt_sb[hsl, q0 + jo:q0 + 512],
                    start=True, stop=True,
                )
            nc.scalar.activation(
                pt_sb[:].rearrange("p (h w) -> p h w", h=2)[:, :, jo:512],
                s_ps[:].rearrange("p (h w) -> p h w", h=2)[:, :, jo:512],
                EXP, scale=float(SCALE),
            )
            if t >= 4 * m:
                # zero the strictly-masked (k > q) triangle of the diagonal
                # 128-block post-exp; replaces a -BIG bias matmul on PE
                ptd = pt_sb[:].rearrange("p (h w) -> p h w", h=2)[:, :, jo:jo + 128]
                trib = tri_sb[:].rearrange("p (o w) -> p o w", o=1)
                nc.vector.tensor_mul(ptd, ptd, trib.broadcast_to([128, 2, 128]))
            pend = state.pop("pend", None)
            if pend is not None:
                av(*pend)
            state["pend"] = (t, pt_sb)
            if t == last_t:
                av(*state.pop("pend"))
                _norm(b, m, z_ps, bcast=("dma" if (b, m) in ((1, 0), (1, 1))
                                         else "pool"))

        for t in range(last_t + 1):
            yield lambda t=t: beat(t)

    def _norm(b, m, z_ps, bcast="pool"):
        # normalize and stage for all-to-all; copy psum out (incl. denom row)
        # immediately to release the z banks, then finish from SBUF
        zcp = [work.tile([65, 512], F32, tag=f"zc{h}", name=f"zc{b}{m}{h}", bufs=2)
               for h in (0, 1)]
        for h in (0, 1):
            nc.vector.tensor_copy(zcp[h][:], z_ps[h][:])
        zt_sb = work.tile([128, 512], BF16, tag="zt", name=f"zt{b}{m}", bufs=2)
        for h in (0, 1):
            recip = work.tile([1, 512], F32, tag="rc", name=f"rc{b}{m}{h}", bufs=2)
            nc.vector.reciprocal(recip[0:1, :], zcp[h][64:65, :])
            bc_sb = work.tile([64, 512], F32, tag="bc", name=f"bcs{b}{m}{h}", bufs=2)
            if bcast == "pool":
                nc.gpsimd.partition_broadcast(bc_sb[:], recip[0:1, :])
            elif bcast == "pe":
                recb = work.tile([1, 512], BF16, tag="rcb", name=f"rb{b}{m}{h}", bufs=2)
                nc.vector.tensor_copy(recb[0:1, :], recip[0:1, :])
                bc_ps = P["ps"].tile([64, 512], F32, tag="s", name=f"bcp{b}{m}{h}")
                nc.tensor.matmul(
                    bc_ps[:], ones_sb[0:1, 0:64], recb[0:1, :],
                    start=True, stop=True,
                )
                nc.vector.tensor_copy(bc_sb[:], bc_ps[:])
            else:
                r_dram = dram.tile([1, 512], F32, tag="rd", name=f"rd{b}{m}{h}", bufs=2)
                nc.sync.dma_start(r_dram[:], recip[0:1, :])
                nc.sync.dma_start(bc_sb[:], r_dram.broadcast_to([64, 512]))
            nc.vector.tensor_mul(
                zt_sb[64 * h:64 * h + 64, :], zcp[h][0:64, :], bc_sb[:]
            )
        nc.sync.dma_start(cc_in[b][2 * m], zt_sb[:, 0:256])
        nc.sync.dma_start(cc_in[b][2 * m + 1], zt_sb[:, 256:512])

    def exchange(i):
        if with_collective:
            nc.gpsimd.collective_compute(
                "AllToAll",
                mybir.AluOpType.bypass,
                replica_groups=[list(range(N_CORES))],
                ins=[cc_in[i].opt()],
                outs=[cc_out[i].opt()],
            )
        else:
            nc.sync.dma_start(cc_out[i][:], cc_in[i][:])

    def load_zt2(i, zt2_sb):
        ccr = cc_out[i].rearrange("j p s -> p j s")
        nc.sync.dma_start(zt2_sb[:, 0:4, :], ccr[:, 0:4, :])
        nc.sync.dma_start(zt2_sb[:, 4:8, :], ccr[:, 4:8, :])

    def out_proj_parts(tag, zt2_sb, sts, rows, pool_fn, ptag=None):
        # O[tok, :] = Z^T.T @ Wo + bo, one closure per 128-token tile
        def part(st, r0):
            def run():
                o_sb = work.tile([128, 1024], F32, tag="o", name=f"os{tag}{st}",
                                 bufs=2)
                for e in (0, 1):
                    o_ps = pool_fn().tile([128, 512], F32,
                                          tag=(ptag or f"o{e}"),
                                          name=f"o{tag}{st}{e}")
                    for i in range(8):
                        nc.tensor.matmul(
                            o_ps[:],
                            zt2_sb[:, i, bass.ts(st, 128)],
                            wo_sb[:, i, bass.ts(e, 512)],
                            start=(i == 0), stop=(i == 7),
                        )
                    nc.vector.tensor_add(
                        o_sb[:, bass.ts(e, 512)], o_ps[:], bo_bc[:, bass.ts(e, 512)]
                    )
                nc.sync.dma_start(out[r0:r0 + 128, :], o_sb[:])
            return run
        return [part(st, r) for st, r in zip(sts, rows)]

    def weave(tasks_a, tasks_b, fillers, boost=0):
        """Round-robin beats from attention streams, sprinkling filler
        closures (projection work) between rounds; `boost` rounds take two
        fillers so early chunks stay ahead of the beats that need them."""
        ia = iter(tasks_a)
        ib = iter(tasks_b)
        fi = iter(fillers)
        done_a = done_b = False
        rnd = 0
        while not (done_a and done_b):
            try:
                next(ia)()
            except StopIteration:
                done_a = True
            try:
                next(ib)()
            except StopIteration:
                done_b = True
            for _ in range(2 if rnd < boost else 1):
                f = next(fi, None)
                if f is not None:
                    f()
            rnd += 1
        for f in fi:
            f()

    def proj_fillers_a():
        for sc in range(1, 8):
            yield from proj_parts(sc)

    # phase A: projections + batch-0 attention (single stream; z uses
    # alternating tag pairs so chunk boundaries overlap)
    with (
        tc.tile_pool(name="ppA", bufs=2, space="PSUM") as ppA,
        tc.tile_pool(name="psA", bufs=2, space="PSUM") as psA,
        tc.tile_pool(name="pzA", bufs=1, space="PSUM") as pzA,
    ):
        P["pp"] = ppA
        P["ps"] = psA
        P["pz"] = pzA
        proj_chunk(0)
        beats_b0 = (
            list(attn_chunk_beats(0, 0, "A")) + list(attn_chunk_beats(0, 1, "A"))
            + list(attn_chunk_beats(0, 2, "A")) + list(attn_chunk_beats(0, 3, "A"))
        )
        weave(beats_b0, [], list(proj_fillers_a()), boost=10)

    # batch-0 Z^T exchange flies while batch-1 attention computes
    exchange(0)

    zt2a = persist.tile([128, 8, 256], BF16, tag="zt2a", name="zt2a")
    zt2b = persist.tile([128, 8, 256], BF16, tag="zt2b", name="zt2b")
    wor = wo.rearrange("p (i e) -> p i e", i=8)

    # phase B: batch-1 attention, two balanced streams
    with (
        tc.tile_pool(name="psB", bufs=2, space="PSUM") as psB,
        tc.tile_pool(name="pzB", bufs=1, space="PSUM") as pzB,
    ):
        P["pp"] = psB
        P["ps"] = psB
        P["pz"] = pzB
        fillers_b = [
            lambda: nc.sync.dma_start(wo_sb[:, 0:4, :], wor[:, 0:4, :]),
            lambda: nc.sync.dma_start(wo_sb[:, 4:8, :], wor[:, 4:8, :]),
            lambda: nc.sync.dma_start(bo_bc[:], bo.broadcast_to([128, D])),
            None, None, None, None, None,
            lambda: load_zt2(0, zt2a),
        ]
        weave(
            list(attn_chunk_beats(1, 0, "A")) + list(attn_chunk_beats(1, 3, "A")),
            list(attn_chunk_beats(1, 1, "B")) + list(attn_chunk_beats(1, 2, "B")),
            fillers_b,
        )

    # batch-1 exchange overlaps batch-0's output projection
    exchange(1)
    with tc.tile_pool(name="poC", bufs=1, space="PSUM") as poC:
        for part in out_proj_parts("a", zt2a, (0, 1), (0, 128), lambda: poC):
            part()
        load_zt2(1, zt2b)
        for part in out_proj_parts("b", zt2b, (0, 1), (256, 384), lambda: poC):
            part()


def _prep_inputs(inputs, Wq, Wk, Wv, Wo, bo):
    bf = ml_dtypes.bfloat16
    x = np.asarray(inputs, dtype=np.float32).reshape(BS, D)
    xT = np.ascontiguousarray(x.T.astype(bf))

    def pec(w):  # [D, C] -> [p, e*C] with D = e*128 + p
        C = w.shape[1]
        return np.ascontiguousarray(
            w.astype(bf).reshape(8, 128, C).transpose(1, 0, 2).reshape(128, 8 * C))

    Wq = np.asarray(Wq, dtype=np.float32)
    Wk = np.asarray(Wk, dtype=np.float32)
    Wv = np.asarray(Wv, dtype=np.float32)
    Wo = pec(np.asarray(Wo, dtype=np.float32))
    bo = np.asarray(bo, dtype=np.float32).reshape(1, D)
    tri01 = np.triu(np.ones((128, 128), dtype=np.float32), k=0).astype(bf)
    ident = np.eye(128, dtype=np.float32).astype(bf)
    onesb = np.ones((128, 128), dtype=np.float32).astype(bf)
    in_maps = []
    for c in range(N_CORES):
        csl = slice(CD * c, CD * (c + 1))
        in_maps.append({
            "xT": xT,
            "wq": pec(Wq[:, csl]),
            "wk": pec(Wk[:, csl]),
            "wv": pec(Wv[:, csl]),
            "wo": Wo,
            "bo": bo,
            "tri01": tri01,
            "ident": ident,
            "onesb": onesb,
        })
    return in_maps


def kernel(inputs, Wq, Wk, Wv, Wo, bo):
    if "nc" not in _CACHE:
        _CACHE["nc"] = build_nc()
    nc = _CACHE["nc"]
    in_maps = _prep_inputs(inputs, Wq, Wk, Wv, Wo, bo)
    res = None
    for attempt in range(3):
        try:
            res = run_bass_kernel_spmd(nc, in_maps, core_ids=list(range(N_CORES)))
            break
        except Exception:
            if attempt == 2:
                raise
            import time as _time

            _time.sleep(5.0)
    # core c owns tokens [256c, 256c+256) of each batch
    full = np.empty((B, S, D), dtype=np.float32)
    for c in range(N_CORES):
        slab = res.results[c]["out"]
        full[0, 256 * c:256 * c + 256] = slab[0:256]
        full[1, 256 * c:256 * c + 256] = slab[256:512]
    return full


# revision 11
# speedup vs baseline: 1.4872x; 1.0376x over previous
"""Multi-head self-attention (B=2, S=2048, D=1024, H=16, causal) on 8 TRN2 cores.

Sharding: tensor-parallel over heads. Core c owns heads {2c, 2c+1}:
  - Wq/Wk/Wv column-sharded: core c gets columns [128c, 128c+128).
  - bf16 datapath end-to-end (PSUM stays f32): x^T, weights, Q^T/K^T/V,
    softmax weights, Z^T all bf16.  Matmuls run 1 cyc/row at any width.
  - Attention in transposed-scores layout: S^T[k, q] tiles; softmax
    denominators come free from a ones-column in V (row 64 of the AV psum).
    Causal masking folded into the scores matmul as a -BIG lower-triangular
    bias matmul on diagonal tiles; fully-masked blocks skipped.
  - Z^T normalized by per-q reciprocal (partition-broadcast on the idle Pool
    engine in phase A; DMA/PE broadcast in phase B to stay off the gpsimd
    queue once collectives are in flight).
  - Two AllToAlls, one per batch: batch-0's Z^T exchange + output-projection
    staging overlap batch-1's attention; only batch-1's exchange + projection
    remain exposed at the tail.  Core j receives all 1024 head-dims for its
    256 tokens of each batch; output projection needs full Wo rows and
    produces a disjoint [512, 1024] output slab per core (no all-reduce).
"""

import ml_dtypes
import numpy as np

import concourse.bass as bass
import concourse.mybir as mybir
import concourse.tile as tile
from concourse import bacc
from concourse.bass_utils import run_bass_kernel_spmd

N_CORES = 8
B, S, D = 2, 2048, 1024
H = 16
HD = D // H          # 64
BS = B * S           # 4096 flattened tokens
CD = 2 * HD          # 128 head-dims per core
BIG = 30000.0
SCALE = 1.0 / np.sqrt(HD)

F32 = mybir.dt.float32
BF16 = mybir.dt.bfloat16
EXP = mybir.ActivationFunctionType.Exp

_CACHE = {}


def build_nc(with_collective=True, reps=1):
    nc = bacc.Bacc("TRN2", target_bir_lowering=False, debug=False, num_devices=N_CORES)

    xT = nc.dram_tensor("xT", [D, BS], BF16, kind="ExternalInput").ap()
    wq = nc.dram_tensor("wq", [D, CD], BF16, kind="ExternalInput").ap()
    wk = nc.dram_tensor("wk", [D, CD], BF16, kind="ExternalInput").ap()
    wv = nc.dram_tensor("wv", [D, CD], BF16, kind="ExternalInput").ap()
    wo = nc.dram_tensor("wo", [D, D], BF16, kind="ExternalInput").ap()
    bo = nc.dram_tensor("bo", [1, D], F32, kind="ExternalInput").ap()
    tri01 = nc.dram_tensor("tri01", [128, 128], BF16, kind="ExternalInput").ap()
    ident = nc.dram_tensor("ident", [128, 128], BF16, kind="ExternalInput").ap()
    onesb = nc.dram_tensor("onesb", [128, 128], BF16, kind="ExternalInput").ap()
    out = nc.dram_tensor("out", [512, D], F32, kind="ExternalOutput").ap()

    with tile.TileContext(nc) as tc:
        with (
            tc.tile_pool(name="const", bufs=1) as constp,
            tc.tile_pool(name="persist", bufs=1) as persist,
            tc.tile_pool(name="xt", bufs=2) as xtp,
            tc.tile_pool(name="work", bufs=3) as work,
            tc.tile_pool(name="dram", bufs=1, space="DRAM") as dram,
        ):
            tri_sb = constp.tile([128, 128], BF16)
            ident_sb = constp.tile([128, 128], BF16)
            ones_sb = constp.tile([128, 128], BF16)
            cc_in = [dram.tile([8, 128, 256], BF16, name=f"ccin{b}") for b in (0, 1)]
            cc_out = [dram.tile([8, 128, 256], BF16, name=f"ccout{b}") for b in (0, 1)]
            xTr = xT.rearrange("(e p) s -> p e s", p=128)

            for _rep in range(reps):
                _body(nc, tc, constp, persist, xtp, work, dram,
                      xTr, wq, wk, wv, wo, bo, out,
                      tri_sb, ident_sb, ones_sb, cc_in, cc_out,
                      with_collective, (tri01, ident, onesb),
                      first=(_rep == 0))

    nc.compile()
    return nc


def _body(nc, tc, constp, persist, xtp, work, dram,
          xTr, wq, wk, wv, wo, bo, out,
          tri_sb, ident_sb, ones_sb, cc_in, cc_out,
          with_collective, const_srcs, first=True):
    # ---- projection weights ----
    wq_sb = constp.tile([128, 8, CD], BF16, tag="wq", name="wq_sb")
    wk_sb = constp.tile([128, 8, CD], BF16, tag="wk", name="wk_sb")
    wv_sb = constp.tile([128, 8, CD], BF16, tag="wv", name="wv_sb")
    wqr = wq.rearrange("(e p) c -> p e c", p=128)
    nc.sync.dma_start(wq_sb[:, 0:1, :], wqr[:, 0:1, :])
    nc.sync.dma_start(wq_sb[:, 1:4, :], wqr[:, 1:4, :])

    # ---- persistent activations ----
    qt_sb = persist.tile([128, BS], BF16, tag="qt", name="qt_sb")
    kt_sb = persist.tile([128, BS], BF16, tag="kt", name="kt_sb")
    v_sb = persist.tile([128, 32, 130], BF16, tag="v", name="v_sb")
    bo_bc = persist.tile([128, D], F32, tag="bobc", name="bo_bc")
    wo_sb = persist.tile([128, 8, D], BF16, tag="wo", name="wo_sb")

    P = {}  # current-phase psum pools

    def v_transposes(sc, vt_t):
        for st in range(4):
            tt = 4 * sc + st
            v_ps = P["pp"].tile([128, 128], BF16, tag="p", name=f"vtp{sc}{st}")
            nc.tensor.transpose(
                v_ps[:], vt_t[:, 128 * st:128 * st + 128], ident_sb[:],
            )
            nc.vector.tensor_copy(v_sb[:, tt, 0:64], v_ps[:, 0:64])
            nc.vector.tensor_copy(v_sb[:, tt, 65:129], v_ps[:, 64:128])

    def proj_parts(sc):
        """Yield fine-grained projection closures for one 512-token chunk."""
        sl = bass.ts(sc, 512)
        state = {}

        def load():
            xt_t = xtp.tile([128, 8, 512], BF16, tag="xt", name=f"xt{sc}")
            if sc == 0:
                nc.sync.dma_start(xt_t[:, 0:1, :], xTr[:, 0:1, sl])
                nc.sync.dma_start(xt_t[:, 1:4, :], xTr[:, 1:4, sl])
                nc.sync.dma_start(wq_sb[:, 4:8, :], wqr[:, 4:8, :])
            else:
                nc.sync.dma_start(xt_t[:, 0:4, :], xTr[:, 0:4, sl])
            nc.sync.dma_start(xt_t[:, 4:8, :], xTr[:, 4:8, sl])
            if sc == 0:
                # defer K/V weight loads so the first Q matmuls start sooner
                nc.sync.dma_start(
                    wk_sb[:], wk.rearrange("(e p) c -> p e c", p=128))
                nc.sync.dma_start(
                    wv_sb[:], wv.rearrange("(e p) c -> p e c", p=128))
                if first:
                    tri_d, ident_d, onesb_d = const_srcs
                    nc.sync.dma_start(tri_sb[:], tri_d)
                    nc.sync.dma_start(ident_sb[:], ident_d)
                    nc.sync.dma_start(ones_sb[:], onesb_d)
                nc.vector.tensor_copy(v_sb[:, :, 64], ones_sb[:, 0:32])
                nc.vector.tensor_copy(v_sb[:, :, 129], ones_sb[:, 0:32])
            state["xt"] = xt_t
            state["vt"] = xtp.tile([128, 512], BF16, tag="vtc", name=f"vtc{sc}")

        def group(w_sb, o_ap_fn, name):
            def run():
                p_ps = P["pp"].tile([128, 512], F32, tag="p", name=f"pp{sc}{name}")
                for e in range(8):
                    nc.tensor.matmul(
                        p_ps[:], w_sb[:, e, :], state["xt"][:, e, :],
                        start=(e == 0), stop=(e == 7),
                    )
                nc.vector.tensor_copy(o_ap_fn(), p_ps[:])
            return run

        yield load
        yield group(wq_sb, lambda: qt_sb[:, sl], "q")
        yield group(wk_sb, lambda: kt_sb[:, sl], "k")
        yield group(wv_sb, lambda: state["vt"][:], "v")
        yield lambda: v_transposes(sc, state["vt"][:])

    def proj_chunk(sc):
        for part in proj_parts(sc):
            part()

    def attn_chunk_beats(b, m, stream):
        """Yield one closure per beat; caller weaves streams together."""
        q0 = 2048 * b + 512 * m
        last_t = 4 * m + 3
        state = {}

        def beat(t):
            if t == 0:
                state["z"] = [
                    P["pz"].tile([65, 512], F32, tag=f"z{stream}{h}",
                                 name=f"z{b}{m}{h}", bufs=1)
                    for h in (0, 1)
                ]
            z_ps = state["z"]

            def av(ta, pt_sb):
                joa = max(0, 128 * (ta - 4 * m))
                for h in (0, 1):
                    nc.tensor.matmul(
                        z_ps[h][:, joa:512],
                        v_sb[:, 16 * b + ta, 65 * h:65 * h + 65],
                        pt_sb[:, 512 * h + joa:512 * h + 512],
                        start=(ta == 0), stop=(ta == last_t),
                    )

            k0 = 2048 * b + 128 * t
            jo = max(0, 128 * (t - 4 * m))
            pt_sb = work.tile([128, 1024], BF16, tag="pt", name=f"pt{b}{m}{t}", bufs=6)
            s_ps = P["ps"].tile([128, 1024], F32, tag="s", name=f"s{b}{m}{t}")
            for h in (0, 1):
                hsl = slice(64 * h, 64 * h + 64)
                nc.tensor.matmul(
                    s_ps[:, 512 * h + jo:512 * h + 512],
                    kt_sb[hsl, k0:k0 + 128],
                    qt_sb[hsl, q0 + jo:q0 + 512],
                    start=True, stop=True,
                )
            nc.scalar.activation(
                pt_sb[:].rearrange("p (h w) -> p h w", h=2)[:, :, jo:512],
                s_ps[:].rearrange("p (h w) -> p h w", h=2)[:, :, jo:512],
                EXP, scale=float(SCALE),
            )
            if t >= 4 * m:
                # zero the strictly-masked (k > q) triangle of the diagonal
                # 128-block post-exp; replaces a -BIG bias matmul on PE
                ptd = pt_sb[:].rearrange("p (h w) -> p h w", h=2)[:, :, jo:jo + 128]
                trib = tri_sb[:].rearrange("p (o w) -> p o w", o=1)
                nc.vector.tensor_mul(ptd, ptd, trib.broadcast_to([128, 2, 128]))
            pend = state.pop("pend", None)
            if pend is not None:
                av(*pend)
            state["pend"] = (t, pt_sb)
            if t == last_t:
                av(*state.pop("pend"))
                _norm(b, m, z_ps, bcast=("dma" if (b, m) in ((1, 0), (1, 1))
                                         else "pool"))

        for t in range(last_t + 1):
            yield lambda t=t: beat(t)

    def _norm(b, m, z_ps, bcast="pool"):
        # normalize and stage for all-to-all; copy psum out (incl. denom row)
        # immediately to release the z banks, then finish from SBUF
        zcp = [work.tile([65, 512], F32, tag=f"zc{h}", name=f"zc{b}{m}{h}", bufs=2)
               for h in (0, 1)]
        for h in (0, 1):
            nc.vector.tensor_copy(zcp[h][:], z_ps[h][:])
        zt_sb = work.tile([128, 512], BF16, tag="zt", name=f"zt{b}{m}", bufs=2)
        for h in (0, 1):
            recip = work.tile([1, 512], F32, tag="rc", name=f"rc{b}{m}{h}", bufs=2)
            nc.vector.reciprocal(recip[0:1, :], zcp[h][64:65, :])
            bc_sb = work.tile([64, 512], F32, tag="bc", name=f"bcs{b}{m}{h}", bufs=2)
            if bcast == "pool":
                nc.gpsimd.partition_broadcast(bc_sb[:], recip[0:1, :])
            elif bcast == "pe":
                recb = work.tile([1, 512], BF16, tag="rcb", name=f"rb{b}{m}{h}", bufs=2)
                nc.vector.tensor_copy(recb[0:1, :], recip[0:1, :])
                bc_ps = P["ps"].tile([64, 512], F32, tag="s", name=f"bcp{b}{m}{h}")
                nc.tensor.matmul(
                    bc_ps[:], ones_sb[0:1, 0:64], recb[0:1, :],
                    start=True, stop=True,
                )
                nc.vector.tensor_copy(bc_sb[:], bc_ps[:])
            else:
                r_dram = dram.tile([1, 512], F32, tag="rd", name=f"rd{b}{m}{h}", bufs=2)
                nc.sync.dma_start(r_dram[:], recip[0:1, :])
                nc.sync.dma_start(bc_sb[:], r_dram.broadcast_to([64, 512]))
            nc.vector.tensor_mul(
                zt_sb[64 * h:64 * h + 64, :], zcp[h][0:64, :], bc_sb[:]
            )
        nc.sync.dma_start(cc_in[b][2 * m], zt_sb[:, 0:256])
        nc.sync.dma_start(cc_in[b][2 * m + 1], zt_sb[:, 256:512])

    def exchange(i):
        if with_collective:
            nc.gpsimd.collective_compute(
                "AllToAll",
                mybir.AluOpType.bypass,
                replica_groups=[list(range(N_CORES))],
                ins=[cc_in[i].opt()],
                outs=[cc_out[i].opt()],
            )
        else:
            nc.sync.dma_start(cc_out[i][:], cc_in[i][:])

    def load_zt2(i, zt2_sb):
        ccr = cc_out[i].rearrange("j p s -> p j s")
        nc.sync.dma_start(zt2_sb[:, 0:4, :], ccr[:, 0:4, :])
        nc.sync.dma_start(zt2_sb[:, 4:8, :], ccr[:, 4:8, :])

    def out_proj_parts(tag, zt2_sb, sts, rows, pool_fn, ptag=None):
        # O[tok, :] = Z^T.T @ Wo + bo, one closure per 128-token tile
        def part(st, r0):
            def run():
                o_sb = work.tile([128, 1024], F32, tag="o", name=f"os{tag}{st}",
                                 bufs=2)
                for e in (0, 1):
                    o_ps = pool_fn().tile([128, 512], F32,
                                          tag=(ptag or f"o{e}"),
                                          name=f"o{tag}{st}{e}")
                    for i in range(8):
                        nc.tensor.matmul(
                            o_ps[:],
                            zt2_sb[:, i, bass.ts(st, 128)],
                            wo_sb[:, i, bass.ts(e, 512)],
                            start=(i == 0), stop=(i == 7),
                        )
                    nc.vector.tensor_add(
                        o_sb[:, bass.ts(e, 512)], o_ps[:], bo_bc[:, bass.ts(e, 512)]
                    )
                nc.sync.dma_start(out[r0:r0 + 128, :], o_sb[:])
            return run
        return [part(st, r) for st, r in zip(sts, rows)]

    def weave(tasks_a, tasks_b, fillers, boost=0):
        """Round-robin beats from attention streams, sprinkling filler
        closures (projection work) between rounds; `boost` rounds take two
        fillers so early chunks stay ahead of the beats that need them."""
        ia = iter(tasks_a)
        ib = iter(tasks_b)
        fi = iter(fillers)
        done_a = done_b = False
        rnd = 0
        while not (done_a and done_b):
            try:
                next(ia)()
            except StopIteration:
                done_a = True
            try:
                next(ib)()
            except StopIteration:
                done_b = True
            for _ in range(2 if rnd < boost else 1):
                f = next(fi, None)
                if f is not None:
                    f()
            rnd += 1
        for f in fi:
            f()

    def proj_fillers_a():
        for sc in range(1, 8):
            yield from proj_parts(sc)

    # phase A: projections + batch-0 attention (single stream; z uses
    # alternating tag pairs so chunk boundaries overlap)
    with (
        tc.tile_pool(name="ppA", bufs=2, space="PSUM") as ppA,
        tc.tile_pool(name="psA", bufs=2, space="PSUM") as psA,
        tc.tile_pool(name="pzA", bufs=1, space="PSUM") as pzA,
    ):
        P["pp"] = ppA
        P["ps"] = psA
        P["pz"] = pzA
        proj_chunk(0)
        beats_b0 = (
            list(attn_chunk_beats(0, 0, "A")) + list(attn_chunk_beats(0, 1, "A"))
            + list(attn_chunk_beats(0, 2, "A")) + list(attn_chunk_beats(0, 3, "A"))
        )
        weave(beats_b0, [], list(proj_fillers_a()), boost=10)

    # batch-0 Z^T exchange flies while batch-1 attention computes
    exchange(0)

    zt2a = persist.tile([128, 8, 256], BF16, tag="zt2a", name="zt2a")
    zt2b = persist.tile([128, 8, 256], BF16, tag="zt2b", name="zt2b")
    wor = wo.rearrange("(i p) e -> p i e", p=128)

    # phase B: batch-1 attention, two balanced streams
    with (
        tc.tile_pool(name="psB", bufs=2, space="PSUM") as psB,
        tc.tile_pool(name="pzB", bufs=1, space="PSUM") as pzB,
    ):
        P["pp"] = psB
        P["ps"] = psB
        P["pz"] = pzB
        fillers_b = [
            lambda: nc.sync.dma_start(wo_sb[:, 0:4, :], wor[:, 0:4, :]),
            lambda: nc.sync.dma_start(wo_sb[:, 4:8, :], wor[:, 4:8, :]),
            lambda: nc.sync.dma_start(bo_bc[:], bo.broadcast_to([128, D])),
            None, None, None, None, None,
            lambda: load_zt2(0, zt2a),
        ]
        weave(
            list(attn_chunk_beats(1, 0, "A")) + list(attn_chunk_beats(1, 3, "A")),
            list(attn_chunk_beats(1, 1, "B")) + list(attn_chunk_beats(1, 2, "B")),
            fillers_b,
        )

    # batch-1 exchange overlaps batch-0's output projection
    exchange(1)
    with tc.tile_pool(name="poC", bufs=1, space="PSUM") as poC:
        for part in out_proj_parts("a", zt2a, (0, 1), (0, 128), lambda: poC):
            part()
        load_zt2(1, zt2b)
        for part in out_proj_parts("b", zt2b, (0, 1), (256, 384), lambda: poC):
            part()


def _prep_inputs(inputs, Wq, Wk, Wv, Wo, bo):
    bf = ml_dtypes.bfloat16
    x = np.asarray(inputs, dtype=np.float32).reshape(BS, D)
    xT = np.ascontiguousarray(x.T.astype(bf))
    Wq = np.asarray(Wq, dtype=np.float32)
    Wk = np.asarray(Wk, dtype=np.float32)
    Wv = np.asarray(Wv, dtype=np.float32)
    Wo = np.ascontiguousarray(np.asarray(Wo, dtype=np.float32).astype(bf))
    bo = np.asarray(bo, dtype=np.float32).reshape(1, D)
    tri01 = np.triu(np.ones((128, 128), dtype=np.float32), k=0).astype(bf)
    ident = np.eye(128, dtype=np.float32).astype(bf)
    onesb = np.ones((128, 128), dtype=np.float32).astype(bf)
    in_maps = []
    for c in range(N_CORES):
        csl = slice(CD * c, CD * (c + 1))
        in_maps.append({
            "xT": xT,
            "wq": np.ascontiguousarray(Wq[:, csl].astype(bf)),
            "wk": np.ascontiguousarray(Wk[:, csl].astype(bf)),
            "wv": np.ascontiguousarray(Wv[:, csl].astype(bf)),
            "wo": Wo,
            "bo": bo,
            "tri01": tri01,
            "ident": ident,
            "onesb": onesb,
        })
    return in_maps


def kernel(inputs, Wq, Wk, Wv, Wo, bo):
    if "nc" not in _CACHE:
        _CACHE["nc"] = build_nc()
    nc = _CACHE["nc"]
    in_maps = _prep_inputs(inputs, Wq, Wk, Wv, Wo, bo)
    res = None
    for attempt in range(3):
        try:
            res = run_bass_kernel_spmd(nc, in_maps, core_ids=list(range(N_CORES)))
            break
        except Exception:
            if attempt == 2:
                raise
            import time as _time

            _time.sleep(5.0)
    # core c owns tokens [256c, 256c+256) of each batch
    full = np.empty((B, S, D), dtype=np.float32)
    for c in range(N_CORES):
        slab = res.results[c]["out"]
        full[0, 256 * c:256 * c + 256] = slab[0:256]
        full[1, 256 * c:256 * c + 256] = slab[256:512]
    return full
